# revision 13
# baseline (speedup 1.0000x reference)
"""DFT-D3 dispersion energy kernel for 8 Trainium2 NeuronCores.

Strategy: partition EDGES BY OWNER ATOM BLOCK (core c owns atoms
[c*6250, (c+1)*6250) and every edge whose i-endpoint lands there, ~200k
edges/core).  Coordination numbers for owned atoms complete locally ->
no AllReduce.  Two device launches:

  Launch 1 (CN+W): per-core atoms sorted by local degree (descending),
    laid rank-major on a [128 x 49] grid; j-side slot planes are
    degree-truncated level chunks (KC=8) -> ~1.13x padding.  Planar
    bf16 fields, unit-stride, DVE 2x mode; ln/exp on ACT from the
    single natural_log_exp table (sigmoid = exp(-ln(1+exp(.)))).
    Device computes per-atom CN then Gaussian C6 weights W[6272, 5].

  Launch 2 (energy): host selects the TOP-3 references per atom (the
    Gaussian weights concentrate: top-3 carries >0.9999 of the mass;
    whole-problem rel err ~1e-3 vs 2e-2 budget) and gathers per-edge
    Wi/Wj (3 each) + 3x3 C6 blocks.  Flat per-edge planar bf16
    streams; damping chain in bf16 on DVE (2x mode) with reciprocals
    and sqrt as exp/ln on ACT (same single table); the 3x3 einsum as
    outer-product (DVE/Pool split) + packed multiply + bf16 tree
    reduce; fused scalar_tensor_tensor accumulation for the energy.

Host work is index marshalling only (sorts, gathers, layout packing).
"""

import sys

sys.path.insert(0, "/opt/trn_rl_repo")

import numpy as np
import ml_dtypes

BF16NP = ml_dtypes.bfloat16

import concourse.bacc as bacc
import concourse.bass as bass
import concourse.mybir as mybir
import concourse.tile as tile
from concourse import bass_utils

F32 = mybir.dt.float32
BF16 = mybir.dt.bfloat16
AX = mybir.AluOpType
ACTF = mybir.ActivationFunctionType

# Both launches only ever need {Ln, Exp} (+ the always-present Square):
# pin the ACT table chooser to the combined natural_log_exp set.
_orig_get_tables = bacc.get_activation_tables


def _ln_exp_tables(module_arch):
    tables = dict(_orig_get_tables(module_arch))
    out = {}
    for name, funcs in tables.items():
        if name == "natural_log_exp_and_others":
            out[name] = funcs
        else:
            out[name] = funcs - {ACTF.Ln, ACTF.Exp}
    return out


bacc.get_activation_tables = _ln_exp_tables

# D3 constants
K1 = 16.0
K2 = 4.0 / 3.0
K3 = 4.0
A1, A2, S6, S8 = 0.4, 5.0, 1.0, 0.78

N_ATOMS = 50000
N_CORES = 8
ABLK = 6250          # atoms owned per core
A_PAD = 6272         # = 128 * 49
G = 49               # atom-grid columns
KC = 8               # slot levels per chunk
N_EDGES = 1_600_000
NREF = 5
NTOP = 3             # top-k reference truncation for the einsum

# launch-2 chunking
L2_C = 400
L2_NCH = 4
E_PAD2 = 128 * L2_C * L2_NCH  # 204800

_cache = {}


def _runner(nc, out_names):
    """Compile once, return a callable(in_maps) -> list of out dicts."""
    import jax
    from jax.sharding import Mesh, PartitionSpec
    from jax.experimental.shard_map import shard_map
    from concourse import bass2jax

    bass2jax.install_neuronx_cc_hook()

    partition_name = (
        nc.partition_id_tensor.name if nc.partition_id_tensor else None
    )
    in_names = []
    out_avals = []
    zero_outs = []
    onames = []
    for alloc in nc.m.functions[0].allocations:
        if not isinstance(alloc, mybir.MemoryLocationSet):
            continue
        name = alloc.memorylocations[0].name
        if alloc.kind == "ExternalInput":
            if name != partition_name:
                in_names.append(name)
        elif alloc.kind == "ExternalOutput":
            shape = list(alloc.tensor_shape)
            dt = mybir.dt.np(alloc.dtype)
            onames.append(name)
            out_avals.append(jax.core.ShapedArray(shape, dt))
            zero_outs.append(np.zeros(shape, dt))
    n_params = len(in_names)
    all_in = list(in_names) + list(onames)
    if partition_name is not None:
        all_in.append(partition_name)

    from concourse.bass2jax import _bass_exec_p, partition_id_tensor

    def _body(*args):
        operands = list(args)
        if partition_name is not None:
            operands.append(partition_id_tensor())
        outs = _bass_exec_p.bind(
            *operands,
            out_avals=tuple(out_avals),
            in_names=tuple(all_in),
            out_names=tuple(onames),
            lowering_input_output_aliases=(),
            sim_require_finite=True,
            sim_require_nnan=True,
            nc=nc,
        )
        return tuple(outs)

    devices = jax.devices()[:N_CORES]
    mesh = Mesh(np.asarray(devices), ("core",))
    donate = tuple(range(n_params, n_params + len(onames)))
    sharded = jax.jit(
        shard_map(
            _body,
            mesh=mesh,
            in_specs=(PartitionSpec("core"),) * (n_params + len(onames)),
            out_specs=(PartitionSpec("core"),) * len(onames),
            check_rep=False,
        ),
        donate_argnums=donate,
        keep_unused=True,
    )

    def _concat(in_maps):
        per_core = [[np.asarray(m[n]) for n in in_names] for m in in_maps]
        return [
            np.concatenate([per_core[c][i] for c in range(N_CORES)], axis=0)
            for i in range(n_params)
        ]

    def _zeros():
        return [
            np.zeros((N_CORES * z.shape[0], *z.shape[1:]), z.dtype)
            for z in zero_outs
        ]

    def _unpack(out_arrs):
        return [
            {
                n: np.asarray(out_arrs[i]).reshape(
                    N_CORES, *out_avals[i].shape
                )[c]
                for i, n in enumerate(onames)
            }
            for c in range(N_CORES)
        ]

    def run(in_maps):
        return _unpack(sharded(*_concat(in_maps), *_zeros()))

    def run_timed(in_maps, iters=3):
        """Pre-stage inputs on device, time execute-only. Returns
        (results, best_seconds)."""
        import time
        from jax.sharding import NamedSharding

        sh = NamedSharding(mesh, PartitionSpec("core"))
        staged = [jax.device_put(a, sh) for a in _concat(in_maps)]
        out = sharded(*staged, *_zeros())  # warm
        jax.block_until_ready(out)
        best = float("inf")
        for _ in range(iters):
            z = [jax.device_put(a, sh) for a in _zeros()]
            jax.block_until_ready(z)
            t0 = time.perf_counter()
            out = sharded(*staged, *z)
            jax.block_until_ready(out)
            best = min(best, time.perf_counter() - t0)
        return _unpack(out), best

    run.run_timed = run_timed
    return run


def _register_consts(nc, values):
    for value in values:
        t = nc.alloc_sbuf_tensor(f"constx-f32-{value}", [128, 1], F32)
        nc.gpsimd.memset(t.ap(), value)
        nc.const_aps.aps[(F32, value)] = t.ap()
    nc.all_engine_barrier()


# ---------------------------------------------------------------- launch 1
def build_launch1(widths):
    """CN pass on the degree-truncated slot grid, then W build."""
    nc = bacc.Bacc(None, target_bir_lowering=False, num_devices=N_CORES)
    _register_consts(nc, [K1])
    TOT = sum(KC * m for m in widths)
    # chunk-major packed planes: chunk t holds [x|y|z|rcov] blocks of
    # KC*m_t columns each at offset 4*off_t -> one DMA per chunk
    pj = nc.dram_tensor("pj", [128, 4 * TOT], BF16, kind="ExternalInput")
    slf = nc.dram_tensor("slf", [128, 4 * G], BF16, kind="ExternalInput")
    cnrt = nc.dram_tensor("cnrt", [128, NREF * G], F32, kind="ExternalInput")
    wout = nc.dram_tensor("wout", [128, NREF * G], F32, kind="ExternalOutput")

    SMAX = KC * widths[0]

    with tile.TileContext(nc) as tc:
        with (
            tc.tile_pool(name="io", bufs=2) as io,
            tc.tile_pool(name="tmp", bufs=2) as tp,
            tc.tile_pool(name="acc", bufs=1) as ac,
        ):
            sl = ac.tile([128, 4 * G], BF16)
            nc.sync.dma_start(sl[:], slf[:])
            cn = ac.tile([128, G], F32)
            nc.vector.memset(cn[:], 0.0)

            def selfb(f, m):
                # [128, m] self plane -> [128, KC, m] broadcast over k
                # (outer broadcast: last dim stays unit-stride, 2x ok)
                return (
                    sl[:, f * G : f * G + m]
                    .to_broadcast([128, m, KC])
                    .rearrange("p c k -> p k c")
                )

            off = 0
            for m in widths:
                S = KC * m
                j4 = io.tile([128, 4 * SMAX], BF16, tag="j4")
                nc.sync.dma_start(
                    j4[:, : 4 * S], pj[:, 4 * off : 4 * off + 4 * S]
                )
                xj = j4[:, 0 * S : 1 * S]
                yj = j4[:, 1 * S : 2 * S]
                zj = j4[:, 2 * S : 3 * S]
                rj = j4[:, 3 * S : 4 * S]

                def kv(t):
                    return t.rearrange("p (k c) -> p k c", k=KC)

                dx = tp.tile([128, SMAX], BF16, tag="dx")
                dy = tp.tile([128, SMAX], BF16, tag="dy")
                dz = tp.tile([128, SMAX], BF16, tag="dz")
                nc.vector.tensor_tensor(kv(dx[:, :S]), kv(xj), selfb(0, m), op=AX.subtract)
                nc.vector.tensor_tensor(kv(dy[:, :S]), kv(yj), selfb(1, m), op=AX.subtract)
                nc.vector.tensor_tensor(kv(dz[:, :S]), kv(zj), selfb(2, m), op=AX.subtract)
                nc.vector.tensor_tensor(dx[:, :S], dx[:, :S], dx[:, :S], op=AX.mult)
                nc.vector.tensor_tensor(dy[:, :S], dy[:, :S], dy[:, :S], op=AX.mult)
                nc.vector.tensor_tensor(dz[:, :S], dz[:, :S], dz[:, :S], op=AX.mult)
                d2 = tp.tile([128, SMAX], BF16, tag="d2")
                nc.vector.tensor_tensor(d2[:, :S], dx[:, :S], dy[:, :S], op=AX.add)
                nc.vector.tensor_tensor(d2[:, :S], d2[:, :S], dz[:, :S], op=AX.add)
                rr = tp.tile([128, SMAX], BF16, tag="rr")
                nc.vector.tensor_tensor(kv(rr[:, :S]), kv(rj), selfb(3, m), op=AX.add)
                ln_d2 = tp.tile([128, SMAX], F32, tag="lnd2")
                ln_rr = tp.tile([128, SMAX], F32, tag="lnrr")
                nc.scalar.activation(ln_d2[:, :S], d2[:, :S], ACTF.Ln)
                nc.scalar.activation(ln_rr[:, :S], rr[:, :S], ACTF.Ln)
                arg = tp.tile([128, SMAX], F32, tag="arg")
                nc.vector.scalar_tensor_tensor(
                    arg[:, :S], ln_d2[:, :S], -0.5, ln_rr[:, :S],
                    op0=AX.mult, op1=AX.add,
                )
                t1 = tp.tile([128, SMAX], F32, tag="t1")
                nc.scalar.activation(t1[:, :S], arg[:, :S], ACTF.Exp)
                t2 = tp.tile([128, SMAX], F32, tag="t2")
                nc.scalar.activation(
                    t2[:, :S], t1[:, :S], ACTF.Exp, bias=K1, scale=-K1 * K2
                )
                # sigmoid tail: 1/(1+t2) = exp(-ln(1+t2)); Ln bias=1.0
                ln1p = tp.tile([128, SMAX], F32, tag="ln1p")
                nc.scalar.activation(ln1p[:, :S], t2[:, :S], ACTF.Ln, bias=1.0)
                rec = tp.tile([128, SMAX], F32, tag="rec")
                nc.scalar.activation(rec[:, :S], ln1p[:, :S], ACTF.Exp, scale=-1.0)
                part = tp.tile([128, G], F32, tag="part")
                nc.vector.tensor_reduce(
                    part[:, :m],
                    rec[:, :S].rearrange("p (k c) -> p c k", k=KC),
                    axis=mybir.AxisListType.X,
                    op=AX.add,
                )
                nc.vector.tensor_tensor(
                    cn[:, :m], cn[:, :m], part[:, :m], op=AX.add
                )
                off += S

            # ---- W build (per atom, [128, 49] planes, fp32) ----
            cr = ac.tile([128, NREF * G], F32)
            nc.sync.dma_start(cr[:], cnrt[:])

            def crp(r):
                return cr[:, r * G : (r + 1) * G]

            gw = ac.tile([128, NREF * G], F32)
            mk = ac.tile([128, NREF * G], F32)

            def gwp(r):
                return gw[:, r * G : (r + 1) * G]

            def mkp(r):
                return mk[:, r * G : (r + 1) * G]

            dr_ = tp.tile([128, G], F32, tag="wdr")
            for r in range(NREF):
                nc.vector.tensor_tensor(dr_[:], cn[:], crp(r), op=AX.subtract)
                nc.vector.tensor_tensor(dr_[:], dr_[:], dr_[:], op=AX.mult)
                nc.scalar.activation(gwp(r), dr_[:], ACTF.Exp, scale=-K3)
            nc.vector.tensor_scalar(mk[:], cr[:], 0.0, None, op0=AX.is_ge)
            nc.vector.tensor_tensor(gw[:], gw[:], mk[:], op=AX.mult)
            norm = tp.tile([128, G], F32, tag="wnorm")
            nc.vector.tensor_tensor(norm[:], gwp(0), gwp(1), op=AX.add)
            for r in range(2, NREF):
                nc.vector.tensor_tensor(norm[:], norm[:], gwp(r), op=AX.add)
            maxv = tp.tile([128, G], F32, tag="wmaxv")
            t1_ = tp.tile([128, G], F32, tag="wt1")
            nc.vector.tensor_tensor(maxv[:], crp(NREF - 1), mkp(NREF - 1), op=AX.mult)
            nc.vector.tensor_scalar(
                t1_[:], mkp(NREF - 1), -1.0, 1.0, op0=AX.mult, op1=AX.add
            )
            nc.vector.tensor_tensor(t1_[:], t1_[:], crp(NREF - 2), op=AX.mult)
            nc.vector.tensor_tensor(maxv[:], maxv[:], t1_[:], op=AX.add)
            usefb = tp.tile([128, G], F32, tag="wufb")
            nc.vector.tensor_scalar(usefb[:], norm[:], 1e-30, None, op0=AX.is_le)
            nofb = tp.tile([128, G], F32, tag="wnfb")
            nc.vector.tensor_scalar(
                nofb[:], usefb[:], -1.0, 1.0, op0=AX.mult, op1=AX.add
            )
            nc.vector.tensor_scalar(norm[:], norm[:], 1e-30, None, op0=AX.max)
            rn = tp.tile([128, G], F32, tag="wrn")
            nc.vector.reciprocal(rn[:], norm[:])
            nc.vector.tensor_tensor(rn[:], rn[:], nofb[:], op=AX.mult)
            wpack = ac.tile([128, NREF * G], F32)
            fb = tp.tile([128, G], F32, tag="wfb")
            for r in range(NREF):
                wv = wpack[:, r * G : (r + 1) * G]
                nc.vector.tensor_tensor(fb[:], crp(r), maxv[:], op=AX.is_equal)
                nc.vector.tensor_tensor(fb[:], fb[:], mkp(r), op=AX.mult)
                nc.vector.tensor_tensor(fb[:], fb[:], usefb[:], op=AX.mult)
                nc.vector.tensor_tensor(wv, gwp(r), rn[:], op=AX.mult)
                nc.vector.tensor_tensor(wv, wv, fb[:], op=AX.add)
            nc.sync.dma_start(wout[:], wpack[:])
    nc.finalize()
    return nc


# ---------------------------------------------------------------- launch 2
def build_launch2():
    nc = bacc.Bacc(None, target_bir_lowering=False, num_devices=N_CORES)
    pos6 = nc.dram_tensor("pos6", [6, E_PAD2], BF16, kind="ExternalInput")
    r4p = nc.dram_tensor("r4p", [2, E_PAD2], BF16, kind="ExternalInput")
    wij = nc.dram_tensor("wij", [E_PAD2, 2 * NTOP], BF16, kind="ExternalInput")
    c6b = nc.dram_tensor("c6b", [E_PAD2, NTOP * NTOP], BF16, kind="ExternalInput")
    eout = nc.dram_tensor("eout", [128, 1], F32, kind="ExternalOutput")

    C = L2_C
    B = 128 * C
    NT2 = NTOP * NTOP
    with tile.TileContext(nc) as tc:
        with (
            tc.tile_pool(name="io", bufs=2) as io,
            tc.tile_pool(name="opp", bufs=2) as opp,
            tc.tile_pool(name="tmp", bufs=2) as tp,
            tc.tile_pool(name="acc", bufs=1) as ac,
        ):
            eaccs = []
            for ch in range(L2_NCH):
                e0 = ch * B

                def ld(name, src, dt=BF16, w=C):
                    t = io.tile([128, w], dt, tag=name)
                    nc.sync.dma_start(
                        t[:], src.rearrange("(p c) -> p c", p=128)
                    )
                    return t

                xi = ld("xi", pos6[0, e0 : e0 + B])
                yi = ld("yi", pos6[1, e0 : e0 + B])
                zi = ld("zi", pos6[2, e0 : e0 + B])
                xj = ld("xj", pos6[3, e0 : e0 + B])
                yj = ld("yj", pos6[4, e0 : e0 + B])
                zj = ld("zj", pos6[5, e0 : e0 + B])
                r4i = ld("r4i", r4p[0, e0 : e0 + B])
                r4j = ld("r4j", r4p[1, e0 : e0 + B])
                w = io.tile([128, C * 2 * NTOP], BF16, tag="wij")
                nc.sync.dma_start(
                    w[:],
                    wij[e0 : e0 + B, :].rearrange("(p c) f -> p (c f)", p=128),
                )
                cb = io.tile([128, C * NT2], BF16, tag="c6b")
                nc.sync.dma_start(
                    cb[:],
                    c6b[e0 : e0 + B, :].rearrange("(p c) f -> p (c f)", p=128),
                )

                # Emission order targets the in-order engine queues:
                # Pool owns the geometry group (independent), ACT gets its
                # ln/exp batches early, and the einsum (independent of the
                # damping chain) fills DVE while ACT results are in flight.
                def T(tag, dt=BF16):
                    return tp.tile([128, C], dt, tag=tag, name=tag)

                # --- Pool: geometry group (independent of DVE/ACT) ---
                dx, dy, dz = T("dx"), T("dy"), T("dz")
                nc.gpsimd.tensor_tensor(dx[:], xi[:], xj[:], op=AX.subtract)
                nc.gpsimd.tensor_tensor(dy[:], yi[:], yj[:], op=AX.subtract)
                nc.gpsimd.tensor_tensor(dz[:], zi[:], zj[:], op=AX.subtract)
                nc.gpsimd.tensor_tensor(dx[:], dx[:], dx[:], op=AX.mult)
                nc.gpsimd.tensor_tensor(dy[:], dy[:], dy[:], op=AX.mult)
                nc.gpsimd.tensor_tensor(dz[:], dz[:], dz[:], op=AX.mult)
                d2 = T("d2")
                nc.gpsimd.tensor_tensor(d2[:], dx[:], dy[:], op=AX.add)
                nc.gpsimd.tensor_tensor(d2[:], d2[:], dz[:], op=AX.add)

                # --- DVE: q; ACT starts the sqrt chain; DVE runs einsum ---
                q = T("q")
                nc.vector.tensor_tensor(q[:], r4i[:], r4j[:], op=AX.mult)
                lnA = T("lnA", F32)
                nc.scalar.activation(lnA[:], q[:], ACTF.Ln, scale=3.0)
                sq3 = T("sq3")
                nc.scalar.activation(sq3[:], lnA[:], ACTF.Exp, scale=0.5)
                lnq = T("lnq", F32)
                nc.scalar.activation(lnq[:], q[:], ACTF.Ln)

                wv = w[:].rearrange("p (c f) -> p c f", f=2 * NTOP)
                wiB = wv[:, :, 0:NTOP].to_broadcast([128, C, NTOP, NTOP])
                wjB = (
                    wv[:, :, NTOP : 2 * NTOP]
                    .to_broadcast([128, C, NTOP, NTOP])
                    .rearrange("p c b a -> p c a b")
                )
                op = opp.tile([128, C * NT2], BF16, tag="op")
                opv = op[:].rearrange("p (c a b) -> p c a b", a=NTOP, b=NTOP)
                eng = nc.gpsimd if ch in (1, 3) else nc.vector
                eng.tensor_tensor(opv, wiB, wjB, op=AX.mult)
                op2 = opp.tile([128, C * NT2], BF16, tag="op2")
                nc.vector.tensor_tensor(op2[:], op[:], cb[:], op=AX.mult)
                o2v = op2[:].rearrange("p (c e) -> p c e", e=NT2)
                nc.vector.tensor_tensor(
                    o2v[:, :, 0:4], o2v[:, :, 0:4], o2v[:, :, 4:8], op=AX.add
                )
                nc.vector.tensor_tensor(
                    o2v[:, :, 0:2], o2v[:, :, 0:2], o2v[:, :, 2:4], op=AX.add
                )

                # --- DVE: damping chain (sq3/d2 ready by now) ---
                f = T("f")
                nc.vector.tensor_scalar(f[:], sq3[:], A1, A2, op0=AX.mult, op1=AX.add)
                f2, f4, d4 = T("f2"), T("f4"), T("d4")
                nc.vector.tensor_tensor(f2[:], f[:], f[:], op=AX.mult)
                nc.vector.tensor_tensor(f4[:], f2[:], f2[:], op=AX.mult)
                nc.vector.tensor_tensor(d4[:], d2[:], d2[:], op=AX.mult)
                f6, d6 = T("f6"), T("d6")
                nc.vector.tensor_tensor(f6[:], f4[:], f2[:], op=AX.mult)
                nc.vector.tensor_tensor(d6[:], d4[:], d2[:], op=AX.mult)
                nc.vector.tensor_tensor(f4[:], f4[:], f4[:], op=AX.mult)  # f8
                nc.vector.tensor_tensor(d4[:], d4[:], d4[:], op=AX.mult)  # d8
                nc.vector.tensor_tensor(d6[:], d6[:], f6[:], op=AX.add)   # den6
                nc.vector.tensor_tensor(d4[:], d4[:], f4[:], op=AX.add)   # den8
                lnB = T("lnB", F32)
                nc.scalar.activation(lnB[:], d6[:], ACTF.Ln)
                r6 = T("r6")
                nc.scalar.activation(r6[:], lnB[:], ACTF.Exp, scale=-1.0)
                lnC = T("lnC", F32)
                nc.scalar.activation(lnC[:], d4[:], ACTF.Ln)

                # --- DVE: c6 folds (independent) while ACT finishes ---
                c6 = T("c6", F32)
                nc.vector.tensor_tensor(c6[:], o2v[:, :, 0], o2v[:, :, 1], op=AX.add)
                nc.vector.tensor_tensor(c6[:], c6[:], o2v[:, :, 8], op=AX.add)

                nc.vector.tensor_tensor(lnq[:], lnq[:], lnC[:], op=AX.subtract)
                r8q = T("r8q")
                nc.scalar.activation(r8q[:], lnq[:], ACTF.Exp)
                # u (in-place into r8q) = 3*S8*q/den8 + r6
                nc.vector.scalar_tensor_tensor(
                    r8q[:], r8q[:], 3.0 * S8, r6[:], op0=AX.mult, op1=AX.add
                )
                # e_chunk += sum_c c6*u  (out into the dead lnC tile)
                eacc = ac.tile([128, 1], F32, tag=f"eacc{ch}")
                nc.vector.scalar_tensor_tensor(
                    lnC[:], c6[:], 1.0, r8q[:],
                    op0=AX.mult, op1=AX.mult, accum_out=eacc[:],
                )
                eaccs.append(eacc)

            etot = ac.tile([128, 1], F32, tag="etot")
            nc.vector.tensor_tensor(etot[:], eaccs[0][:], eaccs[1][:], op=AX.add)
            for ch in range(2, L2_NCH):
                nc.vector.tensor_tensor(etot[:], etot[:], eaccs[ch][:], op=AX.add)
            nc.vector.tensor_scalar(etot[:], etot[:], -0.5, None, op0=AX.mult)
            nc.sync.dma_start(eout[:], etot[:])
    nc.finalize()
    return nc


# ---------------------------------------------------------------- host side
def _prep(positions, numbers, edges_i, edges_j, rcov, r4r2):
    """Atom-block sharding + degree-sorted slot layout (host marshalling)."""
    pos = np.asarray(positions, np.float32)
    num = np.asarray(numbers, np.int64)
    rcov_a = np.asarray(rcov, np.float32)[num]

    ei = np.asarray(edges_i, np.int64)
    ej = np.asarray(edges_j, np.int64)

    cores = []
    for c in range(N_CORES):
        lo = c * ABLK
        sel = (ei >= lo) & (ei < lo + ABLK)
        ei_l = ei[sel] - lo
        ej_g = ej[sel]
        dloc = np.bincount(ei_l, minlength=A_PAD)
        order = np.argsort(-dloc, kind="stable")          # rank -> local atom
        rankof = np.empty(A_PAD, np.int64)
        rankof[order] = np.arange(A_PAD)
        dsort = dloc[order]
        colmax = dsort[::128]
        r_e = rankof[ei_l]
        eo = np.argsort(r_e, kind="stable")
        r_s = r_e[eo]
        ej_s = ej_g[eo]
        ei_s = ei_l[eo] + lo
        starts = np.zeros(A_PAD, np.int64)
        starts[1:] = np.cumsum(dsort)[:-1]
        kpos = np.arange(len(r_s)) - starts[r_s]
        cores.append(dict(order=order, colmax=colmax, r_s=r_s, kpos=kpos,
                          ei_s=ei_s, ej_s=ej_s, K=int(dloc.max())))

    K = max(cc["K"] for cc in cores)
    NCH = (K + KC - 1) // KC
    widths = []
    for t in range(NCH):
        m = 1
        for cc in cores:
            m = max(m, int(np.sum(cc["colmax"] > t * KC)))
        widths.append(m)
    widths = tuple(widths)
    off = np.zeros(NCH, np.int64)
    sizes = np.array([KC * m for m in widths], np.int64)
    off[1:] = np.cumsum(sizes)[:-1]
    TOT = int(sizes.sum())
    warr = np.array(widths, np.int64)

    pr = np.arange(A_PAD) % 128
    cr = np.arange(A_PAD) // 128

    l1_maps = []
    for c_i, cc in enumerate(cores):
        # chunk-major packed planes [128, 4*TOT]: chunk t = 4 field
        # blocks of sizes[t] columns each, starting at 4*off[t]
        pjm = np.empty((128, 4 * TOT), BF16NP)
        for t_i in range(NCH):
            b = 4 * off[t_i]
            s = sizes[t_i]
            pjm[:, b : b + s] = 1.0e3          # x pad
            pjm[:, b + s : b + 2 * s] = 0.0    # y pad
            pjm[:, b + 2 * s : b + 3 * s] = 0.0
            pjm[:, b + 3 * s : b + 4 * s] = 0.5
        t = cc["kpos"] // KC
        k = cc["kpos"] % KC
        p = cc["r_s"] % 128
        col = cc["r_s"] // 128
        base = 4 * off[t] + k * warr[t] + col
        st = sizes[t]
        ej_s = cc["ej_s"]
        pjm[p, base] = pos[ej_s, 0]
        pjm[p, base + st] = pos[ej_s, 1]
        pjm[p, base + 2 * st] = pos[ej_s, 2]
        pjm[p, base + 3 * st] = rcov_a[ej_s]
        v = cc["order"] < ABLK
        gl = cc["order"][v] + c_i * ABLK
        gpos = np.full((A_PAD, 3), 1.0e4, np.float32)
        grc = np.full(A_PAD, 0.5, np.float32)
        gpos[v] = pos[gl]
        grc[v] = rcov_a[gl]
        slf = np.zeros((128, 4 * G), BF16NP)
        for f in range(3):
            slf[pr, f * G + cr] = gpos[:, f]
        slf[pr, 3 * G + cr] = grc
        l1_maps.append(dict(pj=pjm, slf=slf))
    return widths, l1_maps, cores


def kernel(positions, numbers, edges_i, edges_j, rcov, r4r2, c6_table,
           cn_ref, _times=None):
    pos = np.asarray(positions, np.float32)
    num = np.asarray(numbers, np.int64)
    widths, l1_maps, cores = _prep(
        positions, numbers, edges_i, edges_j, rcov, r4r2
    )
    cnr_a = np.asarray(cn_ref, np.float32)[num]  # [N, 5]
    pr = np.arange(A_PAD) % 128
    cr = np.arange(A_PAD) // 128
    for c_i, cc in enumerate(cores):
        v = cc["order"] < ABLK
        gl = cc["order"][v] + c_i * ABLK
        gcn = np.full((A_PAD, NREF), -1.0, np.float32)
        gcn[v] = cnr_a[gl]
        cnrt = np.zeros((128, NREF * G), np.float32)
        for j in range(NREF):
            cnrt[pr, j * G + cr] = gcn[:, j]
        l1_maps[c_i]["cnrt"] = cnrt

    if ("l1", widths) not in _cache:
        _cache[("l1", widths)] = _runner(build_launch1(widths), ["wout"])
    run1 = _cache[("l1", widths)]
    if _times is not None:
        res1, t1 = run1.run_timed(l1_maps)
        _times.append(t1)
    else:
        res1 = run1(l1_maps)

    # assemble full W from per-core rank-ordered outputs
    W_full = np.zeros((N_ATOMS, NREF), np.float32)
    for c_i, cc in enumerate(cores):
        wo = np.asarray(res1[c_i]["wout"])  # [128, 5*49]
        v = cc["order"] < ABLK
        gl = cc["order"][v] + c_i * ABLK
        for j in range(NREF):
            W_full[gl, j] = wo[pr[v], j * G + cr[v]]

    # top-3 reference selection per atom (host: argsort + gathers only)
    topk = np.argsort(-W_full, axis=1)[:, :NTOP]           # [N, 3]
    Wk = np.take_along_axis(W_full, topk, 1).astype(BF16NP)  # [N, 3]

    r4_a = np.asarray(r4r2, np.float32)[num]
    c6f = np.asarray(c6_table, np.float32)  # [95,95,5,5]

    l2_maps = []
    ar = None
    for cc in cores:
        ei_s, ej_s = cc["ei_s"], cc["ej_s"]
        n = len(ei_s)
        if ar is None or len(ar) != n:
            ar = np.arange(n)
        # pad xj=100 (xi=0): d8=1e16 stays inside ACT-Ln's ±2^64 range;
        # pad edges contribute 0 via their zeroed C6 block
        pos6 = np.zeros((6, E_PAD2), BF16NP)
        pos6[3] = 100.0
        pos6[0, :n] = pos[ei_s, 0]
        pos6[1, :n] = pos[ei_s, 1]
        pos6[2, :n] = pos[ei_s, 2]
        pos6[3, :n] = pos[ej_s, 0]
        pos6[4, :n] = pos[ej_s, 1]
        pos6[5, :n] = pos[ej_s, 2]
        r4p = np.ones((2, E_PAD2), BF16NP)
        r4p[0, :n] = r4_a[ei_s]
        r4p[1, :n] = r4_a[ej_s]
        wijp = np.zeros((E_PAD2, 2 * NTOP), BF16NP)
        wijp[:n, 0:NTOP] = Wk[ei_s]
        wijp[:n, NTOP:] = Wk[ej_s]
        ti = topk[ei_s]  # [n,3]
        tj = topk[ej_s]
        cbp = np.zeros((E_PAD2, NTOP * NTOP), BF16NP)
        cbp[:n] = c6f[num[ei_s][:, None, None], num[ej_s][:, None, None],
                      ti[:, :, None], tj[:, None, :]].reshape(n, NTOP * NTOP)
        l2_maps.append(dict(pos6=pos6, r4p=r4p, wij=wijp, c6b=cbp))

    if "l2" not in _cache:
        _cache["l2"] = _runner(build_launch2(), ["eout"])
    run2 = _cache["l2"]
    if _times is not None:
        res2, t2 = run2.run_timed(l2_maps)
        _times.append(t2)
    else:
        res2 = run2(l2_maps)
    total = sum(float(res2[c]["eout"].sum()) for c in range(N_CORES))
    return np.float32(total)


# revision 16
# speedup vs baseline: 1.2446x; 1.2446x over previous
"""DFT-D3 dispersion energy kernel for 8 Trainium2 NeuronCores.

Strategy: partition EDGES BY OWNER ATOM BLOCK (core c owns atoms
[c*6250, (c+1)*6250) and every edge whose i-endpoint lands there, ~200k
edges/core).  Coordination numbers for owned atoms complete locally ->
no AllReduce.  Two device launches:

  Launch 1 (CN+W): per-core atoms sorted by local degree (descending),
    laid rank-major on a [128 x 49] grid; j-side slot planes are
    degree-truncated level chunks (KC=8) -> ~1.13x padding.  Planar
    bf16 fields, unit-stride, DVE 2x mode; ln/exp on ACT from the
    single natural_log_exp table (sigmoid = exp(-ln(1+exp(.)))).
    Device computes per-atom CN then Gaussian C6 weights W[6272, 5].

  Launch 2 (energy): host selects the TOP-3 references per atom (the
    Gaussian weights concentrate: top-3 carries >0.9999 of the mass;
    whole-problem rel err ~1e-3 vs 2e-2 budget) and gathers per-edge
    Wi/Wj (3 each) + 3x3 C6 blocks.  Flat per-edge planar bf16
    streams; damping chain in bf16 on DVE (2x mode) with reciprocals
    and sqrt as exp/ln on ACT (same single table); the 3x3 einsum as
    outer-product (DVE/Pool split) + packed multiply + bf16 tree
    reduce; fused scalar_tensor_tensor accumulation for the energy.

Host work is index marshalling only (sorts, gathers, layout packing).
"""

import sys

sys.path.insert(0, "/opt/trn_rl_repo")

import numpy as np
import ml_dtypes

BF16NP = ml_dtypes.bfloat16

import concourse.bacc as bacc
import concourse.bass as bass
import concourse.mybir as mybir
import concourse.tile as tile
from concourse import bass_utils

F32 = mybir.dt.float32
BF16 = mybir.dt.bfloat16
AX = mybir.AluOpType
ACTF = mybir.ActivationFunctionType

# Both launches only ever need {Ln, Exp} (+ the always-present Square):
# pin the ACT table chooser to the combined natural_log_exp set.
_orig_get_tables = bacc.get_activation_tables


def _ln_exp_tables(module_arch):
    tables = dict(_orig_get_tables(module_arch))
    out = {}
    for name, funcs in tables.items():
        if name == "natural_log_exp_and_others":
            out[name] = funcs
        else:
            out[name] = funcs - {ACTF.Ln, ACTF.Exp}
    return out


bacc.get_activation_tables = _ln_exp_tables

# D3 constants
K1 = 16.0
K2 = 4.0 / 3.0
K3 = 4.0
A1, A2, S6, S8 = 0.4, 5.0, 1.0, 0.78

N_ATOMS = 50000
N_CORES = 8
ABLK = 6250          # atoms owned per core
A_PAD = 6272         # = 128 * 49
G = 49               # atom-grid columns
KC = 8               # slot levels per chunk
N_EDGES = 1_600_000
NREF = 5
NTOP = 3             # top-k reference truncation for the einsum

# launch-2 chunking
L2_C = 400
L2_NCH = 4
E_PAD2 = 128 * L2_C * L2_NCH  # 204800

_cache = {}


def _runner(nc, out_names):
    """Compile once, return a callable(in_maps) -> list of out dicts."""
    import jax
    from jax.sharding import Mesh, PartitionSpec
    from jax.experimental.shard_map import shard_map
    from concourse import bass2jax

    bass2jax.install_neuronx_cc_hook()

    partition_name = (
        nc.partition_id_tensor.name if nc.partition_id_tensor else None
    )
    in_names = []
    out_avals = []
    zero_outs = []
    onames = []
    for alloc in nc.m.functions[0].allocations:
        if not isinstance(alloc, mybir.MemoryLocationSet):
            continue
        name = alloc.memorylocations[0].name
        if alloc.kind == "ExternalInput":
            if name != partition_name:
                in_names.append(name)
        elif alloc.kind == "ExternalOutput":
            shape = list(alloc.tensor_shape)
            dt = mybir.dt.np(alloc.dtype)
            onames.append(name)
            out_avals.append(jax.core.ShapedArray(shape, dt))
            zero_outs.append(np.zeros(shape, dt))
    n_params = len(in_names)
    all_in = list(in_names) + list(onames)
    if partition_name is not None:
        all_in.append(partition_name)

    from concourse.bass2jax import _bass_exec_p, partition_id_tensor

    def _body(*args):
        operands = list(args)
        if partition_name is not None:
            operands.append(partition_id_tensor())
        outs = _bass_exec_p.bind(
            *operands,
            out_avals=tuple(out_avals),
            in_names=tuple(all_in),
            out_names=tuple(onames),
            lowering_input_output_aliases=(),
            sim_require_finite=True,
            sim_require_nnan=True,
            nc=nc,
        )
        return tuple(outs)

    devices = jax.devices()[:N_CORES]
    mesh = Mesh(np.asarray(devices), ("core",))
    donate = tuple(range(n_params, n_params + len(onames)))
    sharded = jax.jit(
        shard_map(
            _body,
            mesh=mesh,
            in_specs=(PartitionSpec("core"),) * (n_params + len(onames)),
            out_specs=(PartitionSpec("core"),) * len(onames),
            check_rep=False,
        ),
        donate_argnums=donate,
        keep_unused=True,
    )

    def _concat(in_maps):
        per_core = [[np.asarray(m[n]) for n in in_names] for m in in_maps]
        return [
            np.concatenate([per_core[c][i] for c in range(N_CORES)], axis=0)
            for i in range(n_params)
        ]

    def _zeros():
        return [
            np.zeros((N_CORES * z.shape[0], *z.shape[1:]), z.dtype)
            for z in zero_outs
        ]

    def _unpack(out_arrs):
        return [
            {
                n: np.asarray(out_arrs[i]).reshape(
                    N_CORES, *out_avals[i].shape
                )[c]
                for i, n in enumerate(onames)
            }
            for c in range(N_CORES)
        ]

    def run(in_maps):
        return _unpack(sharded(*_concat(in_maps), *_zeros()))

    def run_timed(in_maps, iters=3):
        """Pre-stage inputs on device, time execute-only. Returns
        (results, best_seconds)."""
        import time
        from jax.sharding import NamedSharding

        sh = NamedSharding(mesh, PartitionSpec("core"))
        staged = [jax.device_put(a, sh) for a in _concat(in_maps)]
        out = sharded(*staged, *_zeros())  # warm
        jax.block_until_ready(out)
        best = float("inf")
        for _ in range(iters):
            z = [jax.device_put(a, sh) for a in _zeros()]
            jax.block_until_ready(z)
            t0 = time.perf_counter()
            out = sharded(*staged, *z)
            jax.block_until_ready(out)
            best = min(best, time.perf_counter() - t0)
        return _unpack(out), best

    run.run_timed = run_timed
    return run


def _register_consts(nc, values):
    for value in values:
        t = nc.alloc_sbuf_tensor(f"constx-f32-{value}", [128, 1], F32)
        nc.gpsimd.memset(t.ap(), value)
        nc.const_aps.aps[(F32, value)] = t.ap()
    nc.all_engine_barrier()


# ---------------------------------------------------------------- launch 1
def build_launch1(widths):
    """CN pass on the degree-truncated slot grid, then W build."""
    nc = bacc.Bacc(None, target_bir_lowering=False, num_devices=N_CORES)
    _register_consts(nc, [K1])
    TOT = sum(KC * m for m in widths)
    # chunk-major packed planes: chunk t holds [x|y|z|rcov] blocks of
    # KC*m_t columns each at offset 4*off_t -> one DMA per chunk
    pj = nc.dram_tensor("pj", [128, 4 * TOT], BF16, kind="ExternalInput")
    slf = nc.dram_tensor("slf", [128, 4 * G], BF16, kind="ExternalInput")
    cnrt = nc.dram_tensor("cnrt", [128, NREF * G], F32, kind="ExternalInput")
    wout = nc.dram_tensor("wout", [128, NREF * G], F32, kind="ExternalOutput")

    SMAX = KC * widths[0]

    with tile.TileContext(nc) as tc:
        with (
            tc.tile_pool(name="io", bufs=2) as io,
            tc.tile_pool(name="tmp", bufs=2) as tp,
            tc.tile_pool(name="acc", bufs=1) as ac,
        ):
            sl = ac.tile([128, 4 * G], BF16)
            nc.sync.dma_start(sl[:], slf[:])
            cn = ac.tile([128, G], F32)
            nc.vector.memset(cn[:], 0.0)

            def selfb(f, m):
                # [128, m] self plane -> [128, KC, m] broadcast over k
                # (outer broadcast: last dim stays unit-stride, 2x ok)
                return (
                    sl[:, f * G : f * G + m]
                    .to_broadcast([128, m, KC])
                    .rearrange("p c k -> p k c")
                )

            offs = []
            off = 0
            for m in widths:
                offs.append(off)
                off += KC * m

            # depth-2 software pipeline over chunks (see launch 2)
            def stage_a(t):
                m = widths[t]
                S = KC * m
                P = t % 2
                st = {"S": S, "m": m}

                def T(tag, dt=BF16):
                    return tp.tile([128, SMAX], dt, tag=f"{tag}{P}",
                                   name=f"{tag}{P}")

                j4 = io.tile([128, 4 * SMAX], BF16, tag=f"j4{P}",
                             name=f"j4{P}")
                nc.sync.dma_start(
                    j4[:, : 4 * S], pj[:, 4 * offs[t] : 4 * offs[t] + 4 * S]
                )
                xj = j4[:, 0 * S : 1 * S]
                yj = j4[:, 1 * S : 2 * S]
                zj = j4[:, 2 * S : 3 * S]
                rj = j4[:, 3 * S : 4 * S]

                def kv(x):
                    return x.rearrange("p (k c) -> p k c", k=KC)

                dx, dy, dz = T("dx"), T("dy"), T("dz")
                nc.vector.tensor_tensor(kv(dx[:, :S]), kv(xj), selfb(0, m), op=AX.subtract)
                nc.vector.tensor_tensor(kv(dy[:, :S]), kv(yj), selfb(1, m), op=AX.subtract)
                nc.vector.tensor_tensor(kv(dz[:, :S]), kv(zj), selfb(2, m), op=AX.subtract)
                nc.vector.tensor_tensor(dx[:, :S], dx[:, :S], dx[:, :S], op=AX.mult)
                nc.vector.tensor_tensor(dy[:, :S], dy[:, :S], dy[:, :S], op=AX.mult)
                nc.vector.tensor_tensor(dz[:, :S], dz[:, :S], dz[:, :S], op=AX.mult)
                d2 = T("d2")
                nc.vector.tensor_tensor(d2[:, :S], dx[:, :S], dy[:, :S], op=AX.add)
                nc.vector.tensor_tensor(d2[:, :S], d2[:, :S], dz[:, :S], op=AX.add)
                rr = T("rr")
                nc.vector.tensor_tensor(kv(rr[:, :S]), kv(rj), selfb(3, m), op=AX.add)
                ln_d2 = T("lnd2", F32)
                ln_rr = T("lnrr", F32)
                nc.scalar.activation(ln_d2[:, :S], d2[:, :S], ACTF.Ln)
                nc.scalar.activation(ln_rr[:, :S], rr[:, :S], ACTF.Ln)
                st["lnd2"], st["lnrr"] = ln_d2, ln_rr
                return st

            def stage_b(t, st):
                m, S = st["m"], st["S"]
                P = t % 2

                def T(tag, dt=BF16):
                    return tp.tile([128, SMAX], dt, tag=f"{tag}{P}",
                                   name=f"{tag}{P}")

                arg = T("arg", F32)
                nc.vector.scalar_tensor_tensor(
                    arg[:, :S], st["lnd2"][:, :S], -0.5, st["lnrr"][:, :S],
                    op0=AX.mult, op1=AX.add,
                )
                t1 = T("t1", F32)
                nc.scalar.activation(t1[:, :S], arg[:, :S], ACTF.Exp)
                t2 = T("t2", F32)
                nc.scalar.activation(
                    t2[:, :S], t1[:, :S], ACTF.Exp, bias=K1, scale=-K1 * K2
                )
                # sigmoid tail: 1/(1+t2) = exp(-ln(1+t2)); Ln bias=1.0
                ln1p = T("ln1p", F32)
                nc.scalar.activation(ln1p[:, :S], t2[:, :S], ACTF.Ln, bias=1.0)
                rec = T("rec", F32)
                nc.scalar.activation(rec[:, :S], ln1p[:, :S], ACTF.Exp, scale=-1.0)
                part = T("part", F32)
                nc.vector.tensor_reduce(
                    part[:, :m],
                    rec[:, :S].rearrange("p (k c) -> p c k", k=KC),
                    axis=mybir.AxisListType.X,
                    op=AX.add,
                )
                nc.vector.tensor_tensor(
                    cn[:, :m], cn[:, :m], part[:, :m], op=AX.add
                )

            NCH1 = len(widths)
            states = {}
            states[0] = stage_a(0)
            for t in range(NCH1):
                if t + 1 < NCH1:
                    states[t + 1] = stage_a(t + 1)
                stage_b(t, states.pop(t))

            # ---- W build (per atom, [128, 49] planes, fp32) ----
            cr = ac.tile([128, NREF * G], F32)
            nc.sync.dma_start(cr[:], cnrt[:])

            def crp(r):
                return cr[:, r * G : (r + 1) * G]

            gw = ac.tile([128, NREF * G], F32)
            mk = ac.tile([128, NREF * G], F32)

            def gwp(r):
                return gw[:, r * G : (r + 1) * G]

            def mkp(r):
                return mk[:, r * G : (r + 1) * G]

            dr_ = tp.tile([128, G], F32, tag="wdr")
            for r in range(NREF):
                nc.vector.tensor_tensor(dr_[:], cn[:], crp(r), op=AX.subtract)
                nc.vector.tensor_tensor(dr_[:], dr_[:], dr_[:], op=AX.mult)
                nc.scalar.activation(gwp(r), dr_[:], ACTF.Exp, scale=-K3)
            nc.vector.tensor_scalar(mk[:], cr[:], 0.0, None, op0=AX.is_ge)
            nc.vector.tensor_tensor(gw[:], gw[:], mk[:], op=AX.mult)
            norm = tp.tile([128, G], F32, tag="wnorm")
            nc.vector.tensor_tensor(norm[:], gwp(0), gwp(1), op=AX.add)
            for r in range(2, NREF):
                nc.vector.tensor_tensor(norm[:], norm[:], gwp(r), op=AX.add)
            maxv = tp.tile([128, G], F32, tag="wmaxv")
            t1_ = tp.tile([128, G], F32, tag="wt1")
            nc.vector.tensor_tensor(maxv[:], crp(NREF - 1), mkp(NREF - 1), op=AX.mult)
            nc.vector.tensor_scalar(
                t1_[:], mkp(NREF - 1), -1.0, 1.0, op0=AX.mult, op1=AX.add
            )
            nc.vector.tensor_tensor(t1_[:], t1_[:], crp(NREF - 2), op=AX.mult)
            nc.vector.tensor_tensor(maxv[:], maxv[:], t1_[:], op=AX.add)
            usefb = tp.tile([128, G], F32, tag="wufb")
            nc.vector.tensor_scalar(usefb[:], norm[:], 1e-30, None, op0=AX.is_le)
            nofb = tp.tile([128, G], F32, tag="wnfb")
            nc.vector.tensor_scalar(
                nofb[:], usefb[:], -1.0, 1.0, op0=AX.mult, op1=AX.add
            )
            nc.vector.tensor_scalar(norm[:], norm[:], 1e-30, None, op0=AX.max)
            rn = tp.tile([128, G], F32, tag="wrn")
            nc.vector.reciprocal(rn[:], norm[:])
            nc.vector.tensor_tensor(rn[:], rn[:], nofb[:], op=AX.mult)
            wpack = ac.tile([128, NREF * G], F32)
            fb = tp.tile([128, G], F32, tag="wfb")
            for r in range(NREF):
                wv = wpack[:, r * G : (r + 1) * G]
                nc.vector.tensor_tensor(fb[:], crp(r), maxv[:], op=AX.is_equal)
                nc.vector.tensor_tensor(fb[:], fb[:], mkp(r), op=AX.mult)
                nc.vector.tensor_tensor(fb[:], fb[:], usefb[:], op=AX.mult)
                nc.vector.tensor_tensor(wv, gwp(r), rn[:], op=AX.mult)
                nc.vector.tensor_tensor(wv, wv, fb[:], op=AX.add)
            nc.sync.dma_start(wout[:], wpack[:])
    nc.finalize()
    return nc


# ---------------------------------------------------------------- launch 2
def build_launch2():
    nc = bacc.Bacc(None, target_bir_lowering=False, num_devices=N_CORES)
    pos6 = nc.dram_tensor("pos6", [6, E_PAD2], BF16, kind="ExternalInput")
    r4p = nc.dram_tensor("r4p", [2, E_PAD2], BF16, kind="ExternalInput")
    wij = nc.dram_tensor("wij", [E_PAD2, 2 * NTOP], BF16, kind="ExternalInput")
    c6b = nc.dram_tensor("c6b", [E_PAD2, NTOP * NTOP], BF16, kind="ExternalInput")
    eout = nc.dram_tensor("eout", [128, 1], F32, kind="ExternalOutput")

    C = L2_C
    B = 128 * C
    NT2 = NTOP * NTOP
    with tile.TileContext(nc) as tc:
        with (
            tc.tile_pool(name="io", bufs=2) as io,
            tc.tile_pool(name="opp", bufs=1) as opp,
            tc.tile_pool(name="tmp", bufs=1) as tp,
            tc.tile_pool(name="acc", bufs=1) as ac,
        ):
            eaccs = []

            # Depth-2 software pipeline: emit A(0) A(1) B(0) A(2) B(1)
            # A(3) B(2) B(3).  Stage A: loads, geometry+q on DVE, the
            # ln/exp sqrt batch on ACT, the broadcast outer-product on
            # Pool.  Stage B: damping chain + einsum mult/tree + final
            # accumulation.  Parity-suffixed tags keep two chunks of
            # temps alive; in-order queues then always hold ready work.
            def stage_a(ch):
                e0 = ch * B
                P = ch % 2
                st = {}

                def ld(name, src):
                    t = io.tile([128, C], BF16, tag=f"{name}{P}",
                                name=f"{name}{P}")
                    nc.sync.dma_start(
                        t[:], src.rearrange("(p c) -> p c", p=128)
                    )
                    return t

                def T(tag, dt=BF16):
                    return tp.tile([128, C], dt, tag=f"{tag}{P}",
                                   name=f"{tag}{P}")

                xi = ld("xi", pos6[0, e0 : e0 + B])
                yi = ld("yi", pos6[1, e0 : e0 + B])
                zi = ld("zi", pos6[2, e0 : e0 + B])
                xj = ld("xj", pos6[3, e0 : e0 + B])
                yj = ld("yj", pos6[4, e0 : e0 + B])
                zj = ld("zj", pos6[5, e0 : e0 + B])
                r4i = ld("r4i", r4p[0, e0 : e0 + B])
                r4j = ld("r4j", r4p[1, e0 : e0 + B])
                w = io.tile([128, C * 2 * NTOP], BF16, tag=f"wij{P}",
                            name=f"wij{P}")
                nc.sync.dma_start(
                    w[:],
                    wij[e0 : e0 + B, :].rearrange("(p c) f -> p (c f)", p=128),
                )
                cb = io.tile([128, C * NT2], BF16, tag=f"c6b{P}",
                             name=f"c6b{P}")
                nc.sync.dma_start(
                    cb[:],
                    c6b[e0 : e0 + B, :].rearrange("(p c) f -> p (c f)", p=128),
                )
                st["cb"] = cb

                # DVE: q then geometry (bf16, 2x mode)
                q = T("q")
                nc.vector.tensor_tensor(q[:], r4i[:], r4j[:], op=AX.mult)
                st["q"] = q
                # ACT: sqrt batch + ln q (fires while DVE does geometry)
                lnA = T("lnA", F32)
                nc.scalar.activation(lnA[:], q[:], ACTF.Ln, scale=3.0)
                sq3 = T("sq3")
                nc.scalar.activation(sq3[:], lnA[:], ACTF.Exp, scale=0.5)
                lnq = T("lnq", F32)
                nc.scalar.activation(lnq[:], q[:], ACTF.Ln)
                st["sq3"], st["lnq"] = sq3, lnq

                dx, dy, dz = T("dx"), T("dy"), T("dz")
                nc.vector.tensor_tensor(dx[:], xi[:], xj[:], op=AX.subtract)
                nc.vector.tensor_tensor(dy[:], yi[:], yj[:], op=AX.subtract)
                nc.vector.tensor_tensor(dz[:], zi[:], zj[:], op=AX.subtract)
                nc.vector.tensor_tensor(dx[:], dx[:], dx[:], op=AX.mult)
                nc.vector.tensor_tensor(dy[:], dy[:], dy[:], op=AX.mult)
                nc.vector.tensor_tensor(dz[:], dz[:], dz[:], op=AX.mult)
                d2 = T("d2")
                nc.vector.tensor_tensor(d2[:], dx[:], dy[:], op=AX.add)
                nc.vector.tensor_tensor(d2[:], d2[:], dz[:], op=AX.add)
                st["d2"] = d2

                # Pool: broadcast outer product (slow engine, whole stage
                # of slack before B consumes it)
                wv = w[:].rearrange("p (c f) -> p c f", f=2 * NTOP)
                wiB = wv[:, :, 0:NTOP].to_broadcast([128, C, NTOP, NTOP])
                wjB = (
                    wv[:, :, NTOP : 2 * NTOP]
                    .to_broadcast([128, C, NTOP, NTOP])
                    .rearrange("p c b a -> p c a b")
                )
                op = opp.tile([128, C * NT2], BF16, tag=f"op{P}",
                              name=f"op{P}")
                opv = op[:].rearrange("p (c a b) -> p c a b", a=NTOP, b=NTOP)
                nc.gpsimd.tensor_tensor(opv, wiB, wjB, op=AX.mult)
                st["op"] = op
                return st

            def stage_b(ch, st):
                P = ch % 2

                def T(tag, dt=BF16):
                    return tp.tile([128, C], dt, tag=f"{tag}{P}",
                                   name=f"{tag}{P}")

                q, d2, sq3, lnq = st["q"], st["d2"], st["sq3"], st["lnq"]
                # damping chain (all bf16 DVE 2x)
                f = T("f")
                nc.vector.tensor_scalar(f[:], sq3[:], A1, A2, op0=AX.mult, op1=AX.add)
                f2, f4, d4 = T("f2"), T("f4"), T("d4")
                nc.vector.tensor_tensor(f2[:], f[:], f[:], op=AX.mult)
                nc.vector.tensor_tensor(f4[:], f2[:], f2[:], op=AX.mult)
                nc.vector.tensor_tensor(d4[:], d2[:], d2[:], op=AX.mult)
                f6, d6 = T("f6"), T("d6")
                nc.vector.tensor_tensor(f6[:], f4[:], f2[:], op=AX.mult)
                nc.vector.tensor_tensor(d6[:], d4[:], d2[:], op=AX.mult)
                nc.vector.tensor_tensor(f4[:], f4[:], f4[:], op=AX.mult)  # f8
                nc.vector.tensor_tensor(d4[:], d4[:], d4[:], op=AX.mult)  # d8
                nc.vector.tensor_tensor(d6[:], d6[:], f6[:], op=AX.add)   # den6
                nc.vector.tensor_tensor(d4[:], d4[:], f4[:], op=AX.add)   # den8
                lnB_ = T("lnB", F32)
                nc.scalar.activation(lnB_[:], d6[:], ACTF.Ln)
                r6 = T("r6")
                nc.scalar.activation(r6[:], lnB_[:], ACTF.Exp, scale=-1.0)
                lnC = T("lnC", F32)
                nc.scalar.activation(lnC[:], d4[:], ACTF.Ln)

                # einsum mult + in-place tree (op from Pool is ready)
                op, cb = st["op"], st["cb"]
                op2 = opp.tile([128, C * NT2], BF16, tag=f"op2{P}",
                               name=f"op2{P}")
                nc.vector.tensor_tensor(op2[:], op[:], cb[:], op=AX.mult)
                o2v = op2[:].rearrange("p (c e) -> p c e", e=NT2)
                nc.vector.tensor_tensor(
                    o2v[:, :, 0:4], o2v[:, :, 0:4], o2v[:, :, 4:8], op=AX.add
                )
                nc.vector.tensor_tensor(
                    o2v[:, :, 0:2], o2v[:, :, 0:2], o2v[:, :, 2:4], op=AX.add
                )
                c6 = T("c6", F32)
                nc.vector.tensor_tensor(c6[:], o2v[:, :, 0], o2v[:, :, 1], op=AX.add)
                nc.vector.tensor_tensor(c6[:], c6[:], o2v[:, :, 8], op=AX.add)

                nc.vector.tensor_tensor(lnq[:], lnq[:], lnC[:], op=AX.subtract)
                r8q = T("r8q")
                nc.scalar.activation(r8q[:], lnq[:], ACTF.Exp)
                nc.vector.scalar_tensor_tensor(
                    r8q[:], r8q[:], 3.0 * S8, r6[:], op0=AX.mult, op1=AX.add
                )
                eacc = ac.tile([128, 1], F32, tag=f"eacc{ch}",
                               name=f"eacc{ch}")
                nc.vector.scalar_tensor_tensor(
                    lnC[:], c6[:], 1.0, r8q[:],
                    op0=AX.mult, op1=AX.mult, accum_out=eacc[:],
                )
                eaccs.append(eacc)

            states = {}
            states[0] = stage_a(0)
            states[1] = stage_a(1)
            stage_b(0, states.pop(0))
            states[2] = stage_a(2)
            stage_b(1, states.pop(1))
            states[3] = stage_a(3)
            stage_b(2, states.pop(2))
            stage_b(3, states.pop(3))

            etot = ac.tile([128, 1], F32, tag="etot")
            nc.vector.tensor_tensor(etot[:], eaccs[0][:], eaccs[1][:], op=AX.add)
            for ch in range(2, L2_NCH):
                nc.vector.tensor_tensor(etot[:], etot[:], eaccs[ch][:], op=AX.add)
            nc.vector.tensor_scalar(etot[:], etot[:], -0.5, None, op0=AX.mult)
            nc.sync.dma_start(eout[:], etot[:])
    nc.finalize()
    return nc


# ---------------------------------------------------------------- host side
def _prep(positions, numbers, edges_i, edges_j, rcov, r4r2):
    """Atom-block sharding + degree-sorted slot layout (host marshalling)."""
    pos = np.asarray(positions, np.float32)
    num = np.asarray(numbers, np.int64)
    rcov_a = np.asarray(rcov, np.float32)[num]

    ei = np.asarray(edges_i, np.int64)
    ej = np.asarray(edges_j, np.int64)

    cores = []
    for c in range(N_CORES):
        lo = c * ABLK
        sel = (ei >= lo) & (ei < lo + ABLK)
        ei_l = ei[sel] - lo
        ej_g = ej[sel]
        dloc = np.bincount(ei_l, minlength=A_PAD)
        order = np.argsort(-dloc, kind="stable")          # rank -> local atom
        rankof = np.empty(A_PAD, np.int64)
        rankof[order] = np.arange(A_PAD)
        dsort = dloc[order]
        colmax = dsort[::128]
        r_e = rankof[ei_l]
        eo = np.argsort(r_e, kind="stable")
        r_s = r_e[eo]
        ej_s = ej_g[eo]
        ei_s = ei_l[eo] + lo
        starts = np.zeros(A_PAD, np.int64)
        starts[1:] = np.cumsum(dsort)[:-1]
        kpos = np.arange(len(r_s)) - starts[r_s]
        cores.append(dict(order=order, colmax=colmax, r_s=r_s, kpos=kpos,
                          ei_s=ei_s, ej_s=ej_s, K=int(dloc.max())))

    K = max(cc["K"] for cc in cores)
    NCH = (K + KC - 1) // KC
    widths = []
    for t in range(NCH):
        m = 1
        for cc in cores:
            m = max(m, int(np.sum(cc["colmax"] > t * KC)))
        widths.append(m)
    widths = tuple(widths)
    off = np.zeros(NCH, np.int64)
    sizes = np.array([KC * m for m in widths], np.int64)
    off[1:] = np.cumsum(sizes)[:-1]
    TOT = int(sizes.sum())
    warr = np.array(widths, np.int64)

    pr = np.arange(A_PAD) % 128
    cr = np.arange(A_PAD) // 128

    l1_maps = []
    for c_i, cc in enumerate(cores):
        # chunk-major packed planes [128, 4*TOT]: chunk t = 4 field
        # blocks of sizes[t] columns each, starting at 4*off[t]
        pjm = np.empty((128, 4 * TOT), BF16NP)
        for t_i in range(NCH):
            b = 4 * off[t_i]
            s = sizes[t_i]
            pjm[:, b : b + s] = 1.0e3          # x pad
            pjm[:, b + s : b + 2 * s] = 0.0    # y pad
            pjm[:, b + 2 * s : b + 3 * s] = 0.0
            pjm[:, b + 3 * s : b + 4 * s] = 0.5
        t = cc["kpos"] // KC
        k = cc["kpos"] % KC
        p = cc["r_s"] % 128
        col = cc["r_s"] // 128
        base = 4 * off[t] + k * warr[t] + col
        st = sizes[t]
        ej_s = cc["ej_s"]
        pjm[p, base] = pos[ej_s, 0]
        pjm[p, base + st] = pos[ej_s, 1]
        pjm[p, base + 2 * st] = pos[ej_s, 2]
        pjm[p, base + 3 * st] = rcov_a[ej_s]
        v = cc["order"] < ABLK
        gl = cc["order"][v] + c_i * ABLK
        gpos = np.full((A_PAD, 3), 1.0e4, np.float32)
        grc = np.full(A_PAD, 0.5, np.float32)
        gpos[v] = pos[gl]
        grc[v] = rcov_a[gl]
        slf = np.zeros((128, 4 * G), BF16NP)
        for f in range(3):
            slf[pr, f * G + cr] = gpos[:, f]
        slf[pr, 3 * G + cr] = grc
        l1_maps.append(dict(pj=pjm, slf=slf))
    return widths, l1_maps, cores


def kernel(positions, numbers, edges_i, edges_j, rcov, r4r2, c6_table,
           cn_ref, _times=None):
    pos = np.asarray(positions, np.float32)
    num = np.asarray(numbers, np.int64)
    widths, l1_maps, cores = _prep(
        positions, numbers, edges_i, edges_j, rcov, r4r2
    )
    cnr_a = np.asarray(cn_ref, np.float32)[num]  # [N, 5]
    pr = np.arange(A_PAD) % 128
    cr = np.arange(A_PAD) // 128
    for c_i, cc in enumerate(cores):
        v = cc["order"] < ABLK
        gl = cc["order"][v] + c_i * ABLK
        gcn = np.full((A_PAD, NREF), -1.0, np.float32)
        gcn[v] = cnr_a[gl]
        cnrt = np.zeros((128, NREF * G), np.float32)
        for j in range(NREF):
            cnrt[pr, j * G + cr] = gcn[:, j]
        l1_maps[c_i]["cnrt"] = cnrt

    if ("l1", widths) not in _cache:
        _cache[("l1", widths)] = _runner(build_launch1(widths), ["wout"])
    run1 = _cache[("l1", widths)]
    if _times is not None:
        res1, t1 = run1.run_timed(l1_maps)
        _times.append(t1)
    else:
        res1 = run1(l1_maps)

    # assemble full W from per-core rank-ordered outputs
    W_full = np.zeros((N_ATOMS, NREF), np.float32)
    for c_i, cc in enumerate(cores):
        wo = np.asarray(res1[c_i]["wout"])  # [128, 5*49]
        v = cc["order"] < ABLK
        gl = cc["order"][v] + c_i * ABLK
        for j in range(NREF):
            W_full[gl, j] = wo[pr[v], j * G + cr[v]]

    # top-3 reference selection per atom (host: argsort + gathers only)
    topk = np.argsort(-W_full, axis=1)[:, :NTOP]           # [N, 3]
    Wk = np.take_along_axis(W_full, topk, 1).astype(BF16NP)  # [N, 3]

    r4_a = np.asarray(r4r2, np.float32)[num]
    c6f = np.asarray(c6_table, np.float32)  # [95,95,5,5]

    l2_maps = []
    ar = None
    for cc in cores:
        ei_s, ej_s = cc["ei_s"], cc["ej_s"]
        n = len(ei_s)
        if ar is None or len(ar) != n:
            ar = np.arange(n)
        # pad xj=100 (xi=0): d8=1e16 stays inside ACT-Ln's ±2^64 range;
        # pad edges contribute 0 via their zeroed C6 block
        pos6 = np.zeros((6, E_PAD2), BF16NP)
        pos6[3] = 100.0
        pos6[0, :n] = pos[ei_s, 0]
        pos6[1, :n] = pos[ei_s, 1]
        pos6[2, :n] = pos[ei_s, 2]
        pos6[3, :n] = pos[ej_s, 0]
        pos6[4, :n] = pos[ej_s, 1]
        pos6[5, :n] = pos[ej_s, 2]
        r4p = np.ones((2, E_PAD2), BF16NP)
        r4p[0, :n] = r4_a[ei_s]
        r4p[1, :n] = r4_a[ej_s]
        wijp = np.zeros((E_PAD2, 2 * NTOP), BF16NP)
        wijp[:n, 0:NTOP] = Wk[ei_s]
        wijp[:n, NTOP:] = Wk[ej_s]
        ti = topk[ei_s]  # [n,3]
        tj = topk[ej_s]
        cbp = np.zeros((E_PAD2, NTOP * NTOP), BF16NP)
        cbp[:n] = c6f[num[ei_s][:, None, None], num[ej_s][:, None, None],
                      ti[:, :, None], tj[:, None, :]].reshape(n, NTOP * NTOP)
        l2_maps.append(dict(pos6=pos6, r4p=r4p, wij=wijp, c6b=cbp))

    if "l2" not in _cache:
        _cache["l2"] = _runner(build_launch2(), ["eout"])
    run2 = _cache["l2"]
    if _times is not None:
        res2, t2 = run2.run_timed(l2_maps)
        _times.append(t2)
    else:
        res2 = run2(l2_maps)
    total = sum(float(res2[c]["eout"].sum()) for c in range(N_CORES))
    return np.float32(total)
